# revision 13
# baseline (speedup 1.0000x reference)
"""Trainium2 Bass kernel for nn_NeighbourAggregation (gnn_message_passing).

Full-input contract: kernel(states[4096,8] f32, log_tau scalar f32) -> [4096,12] f32.

Strategy (8 cores, shard the query dim i into 8 slices of 512):
  Per query row i the reference reduces algebraically to:
    dist[i,j] = sqrt(|p_i - p_j|^2 + eps),  W = exp(-dist/tau + shift), W[i,i] = 0
    s1 = W @ [pos,vel] / rowsum(W),  s2 = W @ [pos^2,vel^2] / rowsum(W)
    mu = c_i - s1,  sigma = sqrt(s2 - s1^2 + 1e-6)      (i-offsets cancel)
    group_vel = mean(vel),  vel_dev = vel - group_vel
  Device schedule per core (tiles laid out [j=128 partitions, i=512 free]):
    - d2 via PE matmul, fp16 hi/lo split operands (K=10), with +3e-5 injected
      through the |p_i|^2 rank-1 term so d2 > 0 always (no NaN clamp pass)
    - dist = sqrt(d2) on ACT straight from PSUM (sqrt table preloaded at t=0)
    - W = exp(-dist/tau + ln(1000)) on ACT (one table switch; shift cancels in
      the softmax ratio and keeps W in fp16 normal range)
    - diagonal W zeroed by a mask multiply on DVE; per-core j-chunks rotated
      so the diagonal lands in chunks 0..3 (same NEFF on all cores)
    - moments via PE matmul, W fp16 x [Dhi|Dlo] fp16, fp32 PSUM accumulation
    - group_vel: DVE reduce over a host-supplied vel^T/N tile (no PE involved)
    - finalize in transposed layout: one merge+transpose matmul per 128-query
      chunk (lhsT = psM columns, rhs = [I9;I9]), then per-partition-scalar DVE
      ops; sigma sqrt reuses the ACT sqrt table reloaded right after the last
      exp (overlaps the moment-matmul tail)
"""

import sys

sys.path.insert(0, "/opt/trn_rl_repo")

import numpy as np

import concourse.bass as bass
import concourse.mybir as mybir
import concourse.tile as tile
from concourse import bacc
from concourse import bass_utils
from concourse.tile_rust import add_dep_helper

F32 = mybir.dt.float32
F16 = mybir.dt.float16
AF = mybir.ActivationFunctionType
ALU = mybir.AluOpType

N = 4096
NCORES = 8
NI = N // NCORES          # 512 queries per core
P = 128                   # partitions
NCHUNK = N // P           # 32 j-chunks
EXP_SHIFT = float(np.log(1000.0))  # logit shift, cancels in softmax
EPS_BIG = 3e-5            # injected into |p_i|^2 so PE-rounded d2 stays > 0

_BUILT = None


def _build_bass():
    nc = bacc.Bacc(
        "TRN2",
        target_bir_lowering=False,
        debug=False,
        enable_asserts=False,
    )

    def din(name, shape, dt=F32):
        return nc.dram_tensor(name, shape, dt, kind="ExternalInput").ap()

    statj = din("statj", [10, N], F16)
    movi = din("movi", [10, NI], F16)
    dmom = din("dmom", [P, NCHUNK * 18], F16)
    diagmask = din("diagmask", [P, 4 * NI], F16)
    velts = din("velts", [2, N])
    cpack = din("cpack", [P, 24])       # ct4t [.,0:16] + ctvt [.,16:24]
    apack = din("apack", [P, 3])        # actscale, actbias, 1e-6
    selmerge = din("selmerge", [18, 9])  # [I9; I9]
    ones128 = din("ones128", [1, P])
    ident2 = din("ident2", [2, 2])
    out_d = nc.dram_tensor("out", [NI, 12], F32, kind="ExternalOutput").ap()

    with tile.TileContext(nc) as tc:
        with (
            tc.tile_pool(name="consts", bufs=1) as consts,
            tc.tile_pool(name="dist", bufs=1) as distpool,
            tc.tile_pool(name="w", bufs=2) as wpool,
            tc.tile_pool(name="fin", bufs=1) as fin,
        ):
            # ---- load operands (statj/movi gate the start) -------------
            statj_sb = consts.tile([10, N], F16)
            movi_sb = consts.tile([10, NI], F16)
            apack_sb = consts.tile([P, 3], F32)
            velts_sb = consts.tile([2, N], F32)
            dmom_sb = consts.tile([P, NCHUNK * 18], F16)
            diagmask_sb = consts.tile([P, 4 * NI], F16)
            cpack_sb = consts.tile([P, 24], F32)
            selmerge_sb = consts.tile([18, 9], F32)
            ones128_sb = consts.tile([1, P], F32)
            ident2_sb = consts.tile([2, 2], F32)
            # split DMA issue across SP and DVE queues: statj/movi first on SP
            for sb, dr in [
                (statj_sb, statj), (movi_sb, movi), (apack_sb, apack),
                (selmerge_sb, selmerge), (ones128_sb, ones128),
                (ident2_sb, ident2),
            ]:
                nc.sync.dma_start(sb[:], dr[:])
            for sb, dr in [
                (velts_sb, velts), (dmom_sb, dmom), (diagmask_sb, diagmask),
                (cpack_sb, cpack),
            ]:
                nc.gpsimd.dma_start(sb[:], dr[:])

            # trigger the sqrt-table load immediately (no data deps)
            dummy = fin.tile([1, 1], F32, tag="dummy")
            nc.gpsimd.memset(dummy[:], 1.0)
            nc.scalar.activation(dummy[:], dummy[:], AF.Sqrt, bias=0.0)
            # PE warm-up scratch (sets pe_busy_start early so the p-state
            # ramp completes by the time the real matmuls arrive)
            scr16 = fin.tile([1, 1], F16, tag="scr16")
            nc.gpsimd.memset(scr16[:], 1.0)

            # ---- phase A: d2 matmuls -> sqrt from PSUM -----------------
            dist_all = distpool.tile([P, N * 4], F32)   # [128, 16384]
            sqrt_insts = []
            with tc.tile_pool(name="psA", bufs=2, space="PSUM") as psA:
                for h in range(8):
                    ps = psA.tile([P, 2048], F32, tag="psA")
                    if h == 0:
                        # warm-up dummies (overwritten by the start=True
                        # matmuls below)
                        for _ in range(6):
                            nc.tensor.matmul(
                                ps[0:1, 0:1], lhsT=scr16[:], rhs=scr16[:],
                                start=True, stop=True)
                    for q in range(4):
                        t = h * 4 + q
                        nc.tensor.matmul(
                            ps[:, q * NI:(q + 1) * NI],
                            lhsT=statj_sb[:, t * P:(t + 1) * P],
                            rhs=movi_sb[:],
                            start=True,
                            stop=True,
                        )
                    si = nc.scalar.activation(
                        dist_all[:, h * 2048:(h + 1) * 2048],
                        ps[:], AF.Sqrt, bias=0.0)
                    sqrt_insts.append(si)

            # group_vel on DVE, early (velts is pre-scaled by 1/N)
            gvt = fin.tile([2, 1], F32, tag="gvt")
            nc.vector.tensor_reduce(
                out=gvt[:], in_=velts_sb[:], axis=mybir.AxisListType.X,
                op=ALU.add)

            # ---- phase B: exp (table switch), diag mask, moments -------
            psB = tc.tile_pool(name="psB", bufs=1, space="PSUM")
            psBp = psB.__enter__()
            psMa = psBp.tile([18, NI], F32, tag="psMa")
            psMb = psBp.tile([18, NI], F32, tag="psMb")
            last_sqrt = sqrt_insts[-1]
            w_tiles = [wpool.tile([P, N], F16, tag=f"w{g}", name=f"w{g}")
                       for g in range(4)]
            mm_t = 0
            NSPLIT = 24   # chunks 0..23 -> psMa, 24..31 -> psMb

            def moments(w, k):
                nonlocal mm_t
                tgt = psMa if mm_t < NSPLIT else psMb
                nc.tensor.matmul(
                    tgt[:],
                    lhsT=dmom_sb[:, mm_t * 18:(mm_t + 1) * 18],
                    rhs=w[:, k * NI:(k + 1) * NI],
                    start=(mm_t in (0, NSPLIT)),
                    stop=(mm_t in (NSPLIT - 1, NCHUNK - 1)),
                )
                mm_t += 1

            def qexp(g, qq):
                ei = nc.scalar.activation(
                    w_tiles[g][:, qq * 1024:(qq + 1) * 1024],
                    dist_all[:, g * N + qq * 1024: g * N + (qq + 1) * 1024],
                    AF.Exp, bias=apack_sb[:, 1:2], scale=apack_sb[:, 0:1],
                )
                add_dep_helper(ei.ins, last_sqrt.ins, sync=False,
                               reason="exp after all sqrts (table batch)")
                return ei

            # group 0 in quarters: early PE pacing targets + fine moment chase
            e00 = qexp(0, 0)
            # PE pacing dummies across the sqrt->exp table-switch window
            # (written to psMa[0:1,0:1], overwritten by the start=True
            # accumulation below; sync deps pace them so the PE idle gap
            # stays under the ~3us p-state reset threshold)
            d1 = nc.tensor.matmul(psMa[0:1, 0:1], lhsT=scr16[:], rhs=scr16[:],
                                  start=True, stop=True)
            add_dep_helper(d1.ins, last_sqrt.ins, sync=True,
                           reason="PE p-state pacing (after last sqrt)")
            d2 = nc.tensor.matmul(psMa[0:1, 0:1], lhsT=scr16[:], rhs=scr16[:],
                                  start=True, stop=True)
            add_dep_helper(d2.ins, e00.ins, sync=True,
                           reason="PE p-state pacing (after first exp)")
            nc.vector.tensor_tensor(
                out=w_tiles[0][:, 0:1024], in0=w_tiles[0][:, 0:1024],
                in1=diagmask_sb[:, 0:1024], op=ALU.mult)
            moments(w_tiles[0], 0)
            moments(w_tiles[0], 1)
            for qq in range(1, 4):
                qexp(0, qq)
                if qq == 1:
                    nc.vector.tensor_tensor(
                        out=w_tiles[0][:, 1024:2048],
                        in0=w_tiles[0][:, 1024:2048],
                        in1=diagmask_sb[:, 1024:2048], op=ALU.mult)
                moments(w_tiles[0], qq * 2)
                moments(w_tiles[0], qq * 2 + 1)

            for g in (1, 2):
                ei = nc.scalar.activation(
                    w_tiles[g][:], dist_all[:, g * N:(g + 1) * N], AF.Exp,
                    bias=apack_sb[:, 1:2], scale=apack_sb[:, 0:1],
                )
                add_dep_helper(ei.ins, last_sqrt.ins, sync=False,
                               reason="exp after all sqrts (table batch)")
                for k in range(8):
                    moments(w_tiles[g], k)

            # early copy of the chunk-0..23 accumulator (overlaps g3 exps)
            Mall_a = fin.tile([18, NI], F32)
            nc.vector.tensor_copy(Mall_a[:], psMa[:])

            # group 3 in quarters to shorten the tail
            last_exp = None
            for qq in range(4):
                last_exp = qexp(3, qq)
                moments(w_tiles[3], qq * 2)
                moments(w_tiles[3], qq * 2 + 1)

            # reload sqrt table right after the last exp (overlaps moment
            # tail + finalize lead-in; sigma sqrt then costs ~0.2us)
            dummy2 = fin.tile([1, 1], F32, tag="dummy2")
            nc.vector.memset(dummy2[:], 1.0)
            s2i = nc.scalar.activation(dummy2[:], dummy2[:], AF.Sqrt, bias=0.0)
            add_dep_helper(s2i.ins, last_exp.ins, sync=False,
                           reason="sqrt table reload after last exp")

            # ---- finalize (transposed layout) --------------------------
            Mall_b = fin.tile([18, NI], F32)
            nc.vector.tensor_copy(Mall_b[:], psMb[:])
            psB.__exit__(None, None, None)

            psFpool = tc.tile_pool(name="psF", bufs=1, space="PSUM")
            psF = psFpool.__enter__()

            # gv: [2,1] -> [1,2] -> broadcast [128,2]
            psGrow = psF.tile([1, 2], F32, tag="psGrow")
            nc.tensor.transpose(psGrow[:], gvt[:], ident2_sb[:])
            growv = fin.tile([1, 2], F32)
            nc.vector.tensor_copy(growv[:], psGrow[:])
            psGB = psF.tile([P, 2], F32, tag="psGB")
            nc.tensor.matmul(psGB[:], lhsT=ones128_sb[:], rhs=growv[:],
                             start=True, stop=True)
            gvb = fin.tile([P, 2], F32, tag="gvb")
            nc.vector.tensor_copy(gvb[:], psGB[:])

            ot = fin.tile([P, 48], F32, tag="ot")
            sg_all = fin.tile([P, 16], F32, tag="sg")
            ot3 = ot[:].rearrange("p (k d) -> p k d", d=12)
            for k in range(4):
                psT = psF.tile([P, 9], F32, tag=f"psT{k}")
                nc.tensor.matmul(psT[:], lhsT=Mall_a[:, k * P:(k + 1) * P],
                                 rhs=selmerge_sb[:], start=True, stop=False)
                nc.tensor.matmul(psT[:], lhsT=Mall_b[:, k * P:(k + 1) * P],
                                 rhs=selmerge_sb[:], start=False, stop=True)
                rinv = fin.tile([P, 1], F32, tag=f"rinv{k}")
                nc.vector.reciprocal_approx_fast(rinv[:], psT[:, 8:9])
                s_k = fin.tile([P, 8], F32, tag=f"s{k}")
                nc.vector.tensor_scalar(
                    out=s_k[:], in0=psT[:, 0:8], scalar1=rinv[:],
                    scalar2=None, op0=ALU.mult)
                # mu = c - s1  (Pool)
                nc.gpsimd.tensor_tensor(
                    out=ot3[:, k, 0:4], in0=cpack_sb[:, 4 * k:4 * k + 4],
                    in1=s_k[:, 0:4], op=ALU.subtract)
                # sig2 = s2 - s1^2  (DVE)
                t2 = fin.tile([P, 4], F32, tag=f"t2{k}")
                nc.vector.tensor_tensor(out=t2[:], in0=s_k[:, 0:4],
                                        in1=s_k[:, 0:4], op=ALU.mult)
                nc.vector.tensor_tensor(out=sg_all[:, 4 * k:4 * k + 4],
                                        in0=s_k[:, 4:8], in1=t2[:],
                                        op=ALU.subtract)
                # vel_dev + group_vel columns (Pool)
                nc.gpsimd.tensor_tensor(
                    out=ot3[:, k, 10:12],
                    in0=cpack_sb[:, 16 + 2 * k:16 + 2 * k + 2],
                    in1=gvb[:], op=ALU.subtract)
                nc.gpsimd.tensor_copy(ot3[:, k, 8:10], gvb[:])

            # sigma for all 4 chunks in one strided ACT sqrt
            nc.scalar.activation(
                ot3[:, :, 4:8],
                sg_all[:].rearrange("p (k d) -> p k d", d=4),
                AF.Sqrt, bias=apack_sb[:, 2:3])

            out_rr = out_d.rearrange("(k p) d -> p k d", p=P)
            nc.sync.dma_start(out_rr[:], ot3[:])
            psFpool.__exit__(None, None, None)

    nc.finalize()
    return nc


def _host_prep(states, log_tau):
    states = np.asarray(states, dtype=np.float32)
    tau = np.exp(np.float32(log_tau)).astype(np.float32)
    pos = ((states[:, :2] + states[:, 2:4]) / 2.0).astype(np.float32)
    vel = ((states[:, 4:6] + states[:, 6:8]) / 2.0).astype(np.float32)
    p2 = (pos[:, 0] * pos[:, 0] + pos[:, 1] * pos[:, 1]).astype(np.float32)
    p2i = (p2 + np.float32(EPS_BIG)).astype(np.float32)

    f16 = np.float16
    ph = pos.astype(f16)
    pl = (pos - ph.astype(np.float32)).astype(f16)
    p2h = p2.astype(f16)
    p2l = (p2 - p2h.astype(np.float32)).astype(f16)
    p2ih = p2i.astype(f16)
    p2il = (p2i - p2ih.astype(np.float32)).astype(f16)

    C = np.concatenate([pos, vel], axis=1).astype(np.float32)          # [N,4]
    D = np.concatenate([C, C * C, np.ones((N, 1), np.float32)], 1)     # [N,9]
    Dh = D.astype(f16)
    Dl = (D - Dh.astype(np.float32)).astype(f16)

    ones_n = np.ones(N, f16)
    diagmask = np.ones((P, 4 * NI), f16)
    pp = np.arange(P)
    for k in range(4):
        diagmask[pp, k * NI + P * k + pp] = 0.0

    selmerge = np.concatenate([np.eye(9)] * 2, 0).astype(np.float32)
    velts = (vel.T / np.float32(N)).copy().astype(np.float32)          # [2,N]

    in_maps = []
    for c in range(NCORES):
        # j-chunk rotation: device chunk t holds original chunk (t + 4c) % 32
        jperm = np.concatenate(
            [np.arange(((t + 4 * c) % NCHUNK) * P, ((t + 4 * c) % NCHUNK) * P + P)
             for t in range(NCHUNK)]
        )
        isl = np.arange(NI * c, NI * (c + 1))

        statj_a = np.stack([
            ph[jperm, 0], ph[jperm, 1], pl[jperm, 0], pl[jperm, 1],
            ph[jperm, 0], ph[jperm, 1], p2h[jperm], p2l[jperm],
            ones_n[:N], ones_n[:N],
        ]).astype(f16)                                                 # [10, N]
        m2 = np.float16(-2.0)
        movi_a = np.stack([
            m2 * ph[isl, 0], m2 * ph[isl, 1], m2 * ph[isl, 0], m2 * ph[isl, 1],
            m2 * pl[isl, 0], m2 * pl[isl, 1], ones_n[:NI], ones_n[:NI],
            p2ih[isl], p2il[isl],
        ]).astype(f16)                                                 # [10, NI]

        dmom_a = np.empty((P, NCHUNK * 18), f16)
        Dhp = Dh[jperm].reshape(NCHUNK, P, 9)
        Dlp = Dl[jperm].reshape(NCHUNK, P, 9)
        for t in range(NCHUNK):
            dmom_a[:, t * 18:t * 18 + 9] = Dhp[t]
            dmom_a[:, t * 18 + 9:t * 18 + 18] = Dlp[t]

        # transposed per-chunk constants: [128, 16] C and [128, 8] vel
        ct4t = C[isl].reshape(4, P, 4).transpose(1, 0, 2).reshape(P, 16)
        ctvt = vel[isl].reshape(4, P, 2).transpose(1, 0, 2).reshape(P, 8)
        cpack = np.concatenate([ct4t, ctvt], axis=1).astype(np.float32)

        apack = np.stack([
            np.full(P, -1.0 / tau, np.float32),
            np.full(P, EXP_SHIFT, np.float32),
            np.full(P, 1e-6, np.float32),
        ], axis=1)

        in_maps.append({
            "statj": statj_a,
            "movi": movi_a,
            "dmom": dmom_a,
            "diagmask": diagmask,
            "velts": velts,
            "cpack": cpack,
            "apack": apack,
            "selmerge": selmerge,
            "ones128": np.ones((1, P), np.float32),
            "ident2": np.eye(2, dtype=np.float32),
        })
    return in_maps


def _get_built():
    global _BUILT
    if _BUILT is None:
        _BUILT = _build_bass()
    return _BUILT


def kernel(states, log_tau, _trace=False, _trace_kwargs=None):
    nc = _get_built()
    in_maps = _host_prep(states, log_tau)
    res = bass_utils.run_bass_kernel_spmd(
        nc, in_maps, core_ids=list(range(NCORES)),
        trace=_trace, **(_trace_kwargs or {}),
    )
    out = np.concatenate([res.results[c]["out"] for c in range(NCORES)], axis=0)
    if _trace:
        kernel._last_results = res
    return out.astype(np.float32)


# revision 14
# speedup vs baseline: 1.1022x; 1.1022x over previous
"""Trainium2 Bass kernel for nn_NeighbourAggregation (gnn_message_passing).

Full-input contract: kernel(states[4096,8] f32, log_tau scalar f32) -> [4096,12] f32.

Strategy (8 cores, shard the query dim i into 8 slices of 512):
  Per query row i the reference reduces algebraically to:
    dist[i,j] = sqrt(|p_i - p_j|^2 + eps),  W = exp(-dist/tau + shift), W[i,i] = 0
    s1 = W @ [pos,vel] / rowsum(W),  s2 = W @ [pos^2,vel^2] / rowsum(W)
    mu = c_i - s1,  sigma = sqrt(s2 - s1^2 + 1e-6)      (i-offsets cancel)
    group_vel = mean(vel),  vel_dev = vel - group_vel
  Device schedule per core (tiles laid out [j=128 partitions, i=512 free]):
    - d2 via PE matmul, fp16 hi/lo split operands (K=10), with +3e-5 injected
      through the |p_i|^2 rank-1 term so d2 > 0 always (no NaN clamp pass)
    - dist = sqrt(d2) on ACT straight from PSUM (sqrt table preloaded at t=0)
    - W = exp(-dist/tau + ln(1000)) on ACT (one table switch; shift cancels in
      the softmax ratio and keeps W in fp16 normal range)
    - diagonal W zeroed by a mask multiply on DVE; per-core j-chunks rotated
      so the diagonal lands in chunks 0..3 (same NEFF on all cores)
    - moments via PE matmul, W fp16 x [Dhi|Dlo] fp16, fp32 PSUM accumulation
    - group_vel: DVE reduce over a host-supplied vel^T/N tile (no PE involved)
    - finalize in transposed layout: one merge+transpose matmul per 128-query
      chunk (lhsT = psM columns, rhs = [I9;I9]), then per-partition-scalar DVE
      ops; sigma sqrt reuses the ACT sqrt table reloaded right after the last
      exp (overlaps the moment-matmul tail)
"""

import sys

sys.path.insert(0, "/opt/trn_rl_repo")

import numpy as np

import concourse.bass as bass
import concourse.mybir as mybir
import concourse.tile as tile
from concourse import bacc
from concourse import bass_utils
from concourse.tile_rust import add_dep_helper

F32 = mybir.dt.float32
F16 = mybir.dt.float16
AF = mybir.ActivationFunctionType
ALU = mybir.AluOpType

N = 4096
NCORES = 8
NI = N // NCORES          # 512 queries per core
P = 128                   # partitions
NCHUNK = N // P           # 32 j-chunks
EXP_SHIFT = float(np.log(1000.0))  # logit shift, cancels in softmax
EPS_BIG = 3e-5            # injected into |p_i|^2 so PE-rounded d2 stays > 0

_BUILT = None


def _build_bass():
    nc = bacc.Bacc(
        "TRN2",
        target_bir_lowering=False,
        debug=False,
        enable_asserts=False,
    )

    def din(name, shape, dt=F32):
        return nc.dram_tensor(name, shape, dt, kind="ExternalInput").ap()

    statj = din("statj", [10, N], F16)
    movi = din("movi", [10, NI], F16)
    dmom = din("dmom", [P, NCHUNK * 18], F16)
    diagmask = din("diagmask", [P, 4 * NI], F16)
    velts = din("velts", [2, N])
    cpack = din("cpack", [P, 24])       # ct4t [.,0:16] + ctvt [.,16:24]
    apack = din("apack", [P, 3])        # actscale, actbias, 1e-6
    selmerge = din("selmerge", [18, 9])  # [I9; I9]
    ones128 = din("ones128", [1, P])
    ident2 = din("ident2", [2, 2])
    out_d = nc.dram_tensor("out", [NI, 12], F32, kind="ExternalOutput").ap()

    with tile.TileContext(nc) as tc:
        with (
            tc.tile_pool(name="consts", bufs=1) as consts,
            tc.tile_pool(name="dist", bufs=1) as distpool,
            tc.tile_pool(name="w", bufs=2) as wpool,
            tc.tile_pool(name="fin", bufs=1) as fin,
        ):
            # ---- load operands (statj/movi gate the start) -------------
            statj_sb = consts.tile([10, N], F16)
            movi_sb = consts.tile([10, NI], F16)
            apack_sb = consts.tile([P, 3], F32)
            velts_sb = consts.tile([2, N], F32)
            dmom_sb = consts.tile([P, NCHUNK * 18], F16)
            diagmask_sb = consts.tile([P, 4 * NI], F16)
            cpack_sb = consts.tile([P, 24], F32)
            selmerge_sb = consts.tile([18, 9], F32)
            ones128_sb = consts.tile([1, P], F32)
            ident2_sb = consts.tile([2, 2], F32)
            # scratch memsets first so the PE warm-up and sqrt-table
            # trigger fire within the first ~0.3us
            dummy = fin.tile([1, 1], F32, tag="dummy")
            nc.gpsimd.memset(dummy[:], 1.0)
            scr16 = fin.tile([1, 1], F16, tag="scr16")
            nc.gpsimd.memset(scr16[:], 1.0)
            nc.scalar.activation(dummy[:], dummy[:], AF.Sqrt, bias=0.0)
            # input DMAs on the SP queue, most-urgent first
            for sb, dr in [
                (statj_sb, statj), (movi_sb, movi), (apack_sb, apack),
                (velts_sb, velts), (dmom_sb, dmom), (diagmask_sb, diagmask),
                (cpack_sb, cpack), (selmerge_sb, selmerge),
                (ones128_sb, ones128), (ident2_sb, ident2),
            ]:
                nc.sync.dma_start(sb[:], dr[:])

            # ---- phase A: d2 matmuls -> sqrt from PSUM -----------------
            dist_all = distpool.tile([P, N * 4], F32)   # [128, 16384]
            sqrt_insts = []
            with tc.tile_pool(name="psA", bufs=2, space="PSUM") as psA:
                for h in range(8):
                    ps = psA.tile([P, 2048], F32, tag="psA")
                    if h == 0:
                        # warm-up dummies (overwritten by the start=True
                        # matmuls below)
                        for _ in range(6):
                            nc.tensor.matmul(
                                ps[0:1, 0:1], lhsT=scr16[:], rhs=scr16[:],
                                start=True, stop=True)
                    for q in range(4):
                        t = h * 4 + q
                        nc.tensor.matmul(
                            ps[:, q * NI:(q + 1) * NI],
                            lhsT=statj_sb[:, t * P:(t + 1) * P],
                            rhs=movi_sb[:],
                            start=True,
                            stop=True,
                        )
                    si = nc.scalar.activation(
                        dist_all[:, h * 2048:(h + 1) * 2048],
                        ps[:], AF.Sqrt, bias=0.0)
                    sqrt_insts.append(si)

            # group_vel on DVE, early (velts is pre-scaled by 1/N)
            gvt = fin.tile([2, 1], F32, tag="gvt")
            nc.vector.tensor_reduce(
                out=gvt[:], in_=velts_sb[:], axis=mybir.AxisListType.X,
                op=ALU.add)

            # ---- phase B: exp (table switch), diag mask, moments -------
            psB = tc.tile_pool(name="psB", bufs=1, space="PSUM")
            psBp = psB.__enter__()
            psMa = psBp.tile([18, NI], F32, tag="psMa")
            psMb = psBp.tile([18, NI], F32, tag="psMb")
            last_sqrt = sqrt_insts[-1]
            w_tiles = [wpool.tile([P, N], F16, tag=f"w{g}", name=f"w{g}")
                       for g in range(4)]
            mm_t = 0
            NSPLIT = 24   # chunks 0..23 -> psMa, 24..31 -> psMb

            def moments(w, k):
                nonlocal mm_t
                tgt = psMa if mm_t < NSPLIT else psMb
                nc.tensor.matmul(
                    tgt[:],
                    lhsT=dmom_sb[:, mm_t * 18:(mm_t + 1) * 18],
                    rhs=w[:, k * NI:(k + 1) * NI],
                    start=(mm_t in (0, NSPLIT)),
                    stop=(mm_t in (NSPLIT - 1, NCHUNK - 1)),
                )
                mm_t += 1

            def qexp(g, qq):
                ei = nc.scalar.activation(
                    w_tiles[g][:, qq * 1024:(qq + 1) * 1024],
                    dist_all[:, g * N + qq * 1024: g * N + (qq + 1) * 1024],
                    AF.Exp, bias=apack_sb[:, 1:2], scale=apack_sb[:, 0:1],
                )
                add_dep_helper(ei.ins, last_sqrt.ins, sync=False,
                               reason="exp after all sqrts (table batch)")
                return ei

            # group 0 in quarters: early PE pacing targets + fine moment chase
            e00 = qexp(0, 0)
            # PE pacing dummies across the sqrt->exp table-switch window
            # (written to psMa[0:1,0:1], overwritten by the start=True
            # accumulation below; sync deps pace them so the PE idle gap
            # stays under the ~3us p-state reset threshold)
            d1 = nc.tensor.matmul(psMa[0:1, 0:1], lhsT=scr16[:], rhs=scr16[:],
                                  start=True, stop=True)
            add_dep_helper(d1.ins, last_sqrt.ins, sync=True,
                           reason="PE p-state pacing (after last sqrt)")
            d2 = nc.tensor.matmul(psMa[0:1, 0:1], lhsT=scr16[:], rhs=scr16[:],
                                  start=True, stop=True)
            add_dep_helper(d2.ins, e00.ins, sync=True,
                           reason="PE p-state pacing (after first exp)")
            nc.vector.tensor_tensor(
                out=w_tiles[0][:, 0:1024], in0=w_tiles[0][:, 0:1024],
                in1=diagmask_sb[:, 0:1024], op=ALU.mult)
            moments(w_tiles[0], 0)
            moments(w_tiles[0], 1)
            for qq in range(1, 4):
                qexp(0, qq)
                if qq == 1:
                    nc.vector.tensor_tensor(
                        out=w_tiles[0][:, 1024:2048],
                        in0=w_tiles[0][:, 1024:2048],
                        in1=diagmask_sb[:, 1024:2048], op=ALU.mult)
                moments(w_tiles[0], qq * 2)
                moments(w_tiles[0], qq * 2 + 1)

            for g in (1, 2):
                ei = nc.scalar.activation(
                    w_tiles[g][:], dist_all[:, g * N:(g + 1) * N], AF.Exp,
                    bias=apack_sb[:, 1:2], scale=apack_sb[:, 0:1],
                )
                add_dep_helper(ei.ins, last_sqrt.ins, sync=False,
                               reason="exp after all sqrts (table batch)")
                for k in range(8):
                    moments(w_tiles[g], k)

            # early copy of the chunk-0..23 accumulator (overlaps g3 exps)
            Mall_a = fin.tile([18, NI], F32)
            nc.vector.tensor_copy(Mall_a[:], psMa[:])

            # group 3 in quarters to shorten the tail
            last_exp = None
            for qq in range(4):
                last_exp = qexp(3, qq)
                moments(w_tiles[3], qq * 2)
                moments(w_tiles[3], qq * 2 + 1)

            # reload sqrt table right after the last exp (overlaps moment
            # tail + finalize lead-in; sigma sqrt then costs ~0.2us)
            dummy2 = fin.tile([1, 1], F32, tag="dummy2")
            nc.vector.memset(dummy2[:], 1.0)
            s2i = nc.scalar.activation(dummy2[:], dummy2[:], AF.Sqrt, bias=0.0)
            add_dep_helper(s2i.ins, last_exp.ins, sync=False,
                           reason="sqrt table reload after last exp")

            # ---- finalize (transposed layout) --------------------------
            Mall_b = fin.tile([18, NI], F32)
            nc.vector.tensor_copy(Mall_b[:], psMb[:])
            psB.__exit__(None, None, None)

            psFpool = tc.tile_pool(name="psF", bufs=1, space="PSUM")
            psF = psFpool.__enter__()

            # gv: [2,1] -> [1,2] -> broadcast [128,2]
            psGrow = psF.tile([1, 2], F32, tag="psGrow")
            nc.tensor.transpose(psGrow[:], gvt[:], ident2_sb[:])
            growv = fin.tile([1, 2], F32)
            nc.vector.tensor_copy(growv[:], psGrow[:])
            psGB = psF.tile([P, 2], F32, tag="psGB")
            nc.tensor.matmul(psGB[:], lhsT=ones128_sb[:], rhs=growv[:],
                             start=True, stop=True)
            gvb = fin.tile([P, 2], F32, tag="gvb")
            nc.vector.tensor_copy(gvb[:], psGB[:])

            ot = fin.tile([P, 48], F32, tag="ot")
            sg_all = fin.tile([P, 16], F32, tag="sg")
            ot3 = ot[:].rearrange("p (k d) -> p k d", d=12)
            for k in range(4):
                psT = psF.tile([P, 9], F32, tag=f"psT{k}")
                nc.tensor.matmul(psT[:], lhsT=Mall_a[:, k * P:(k + 1) * P],
                                 rhs=selmerge_sb[:], start=True, stop=False)
                nc.tensor.matmul(psT[:], lhsT=Mall_b[:, k * P:(k + 1) * P],
                                 rhs=selmerge_sb[:], start=False, stop=True)
                rinv = fin.tile([P, 1], F32, tag=f"rinv{k}")
                nc.vector.reciprocal_approx_fast(rinv[:], psT[:, 8:9])
                s_k = fin.tile([P, 8], F32, tag=f"s{k}")
                nc.vector.tensor_scalar(
                    out=s_k[:], in0=psT[:, 0:8], scalar1=rinv[:],
                    scalar2=None, op0=ALU.mult)
                # mu = c - s1  (Pool)
                nc.gpsimd.tensor_tensor(
                    out=ot3[:, k, 0:4], in0=cpack_sb[:, 4 * k:4 * k + 4],
                    in1=s_k[:, 0:4], op=ALU.subtract)
                # sig2 = s2 - s1^2  (DVE)
                t2 = fin.tile([P, 4], F32, tag=f"t2{k}")
                nc.vector.tensor_tensor(out=t2[:], in0=s_k[:, 0:4],
                                        in1=s_k[:, 0:4], op=ALU.mult)
                nc.vector.tensor_tensor(out=sg_all[:, 4 * k:4 * k + 4],
                                        in0=s_k[:, 4:8], in1=t2[:],
                                        op=ALU.subtract)
                # vel_dev + group_vel columns (Pool)
                nc.gpsimd.tensor_tensor(
                    out=ot3[:, k, 10:12],
                    in0=cpack_sb[:, 16 + 2 * k:16 + 2 * k + 2],
                    in1=gvb[:], op=ALU.subtract)
                nc.gpsimd.tensor_copy(ot3[:, k, 8:10], gvb[:])

            # sigma for all 4 chunks in one strided ACT sqrt
            nc.scalar.activation(
                ot3[:, :, 4:8],
                sg_all[:].rearrange("p (k d) -> p k d", d=4),
                AF.Sqrt, bias=apack_sb[:, 2:3])

            out_rr = out_d.rearrange("(k p) d -> p k d", p=P)
            nc.sync.dma_start(out_rr[:], ot3[:])
            psFpool.__exit__(None, None, None)

    nc.finalize()
    return nc


def _host_prep(states, log_tau):
    states = np.asarray(states, dtype=np.float32)
    tau = np.exp(np.float32(log_tau)).astype(np.float32)
    pos = ((states[:, :2] + states[:, 2:4]) / 2.0).astype(np.float32)
    vel = ((states[:, 4:6] + states[:, 6:8]) / 2.0).astype(np.float32)
    p2 = (pos[:, 0] * pos[:, 0] + pos[:, 1] * pos[:, 1]).astype(np.float32)
    p2i = (p2 + np.float32(EPS_BIG)).astype(np.float32)

    f16 = np.float16
    ph = pos.astype(f16)
    pl = (pos - ph.astype(np.float32)).astype(f16)
    p2h = p2.astype(f16)
    p2l = (p2 - p2h.astype(np.float32)).astype(f16)
    p2ih = p2i.astype(f16)
    p2il = (p2i - p2ih.astype(np.float32)).astype(f16)

    C = np.concatenate([pos, vel], axis=1).astype(np.float32)          # [N,4]
    D = np.concatenate([C, C * C, np.ones((N, 1), np.float32)], 1)     # [N,9]
    Dh = D.astype(f16)
    Dl = (D - Dh.astype(np.float32)).astype(f16)

    ones_n = np.ones(N, f16)
    diagmask = np.ones((P, 4 * NI), f16)
    pp = np.arange(P)
    for k in range(4):
        diagmask[pp, k * NI + P * k + pp] = 0.0

    selmerge = np.concatenate([np.eye(9)] * 2, 0).astype(np.float32)
    velts = (vel.T / np.float32(N)).copy().astype(np.float32)          # [2,N]

    in_maps = []
    for c in range(NCORES):
        # j-chunk rotation: device chunk t holds original chunk (t + 4c) % 32
        jperm = np.concatenate(
            [np.arange(((t + 4 * c) % NCHUNK) * P, ((t + 4 * c) % NCHUNK) * P + P)
             for t in range(NCHUNK)]
        )
        isl = np.arange(NI * c, NI * (c + 1))

        statj_a = np.stack([
            ph[jperm, 0], ph[jperm, 1], pl[jperm, 0], pl[jperm, 1],
            ph[jperm, 0], ph[jperm, 1], p2h[jperm], p2l[jperm],
            ones_n[:N], ones_n[:N],
        ]).astype(f16)                                                 # [10, N]
        m2 = np.float16(-2.0)
        movi_a = np.stack([
            m2 * ph[isl, 0], m2 * ph[isl, 1], m2 * ph[isl, 0], m2 * ph[isl, 1],
            m2 * pl[isl, 0], m2 * pl[isl, 1], ones_n[:NI], ones_n[:NI],
            p2ih[isl], p2il[isl],
        ]).astype(f16)                                                 # [10, NI]

        dmom_a = np.empty((P, NCHUNK * 18), f16)
        Dhp = Dh[jperm].reshape(NCHUNK, P, 9)
        Dlp = Dl[jperm].reshape(NCHUNK, P, 9)
        for t in range(NCHUNK):
            dmom_a[:, t * 18:t * 18 + 9] = Dhp[t]
            dmom_a[:, t * 18 + 9:t * 18 + 18] = Dlp[t]

        # transposed per-chunk constants: [128, 16] C and [128, 8] vel
        ct4t = C[isl].reshape(4, P, 4).transpose(1, 0, 2).reshape(P, 16)
        ctvt = vel[isl].reshape(4, P, 2).transpose(1, 0, 2).reshape(P, 8)
        cpack = np.concatenate([ct4t, ctvt], axis=1).astype(np.float32)

        apack = np.stack([
            np.full(P, -1.0 / tau, np.float32),
            np.full(P, EXP_SHIFT, np.float32),
            np.full(P, 1e-6, np.float32),
        ], axis=1)

        in_maps.append({
            "statj": statj_a,
            "movi": movi_a,
            "dmom": dmom_a,
            "diagmask": diagmask,
            "velts": velts,
            "cpack": cpack,
            "apack": apack,
            "selmerge": selmerge,
            "ones128": np.ones((1, P), np.float32),
            "ident2": np.eye(2, dtype=np.float32),
        })
    return in_maps


def _get_built():
    global _BUILT
    if _BUILT is None:
        _BUILT = _build_bass()
    return _BUILT


def kernel(states, log_tau, _trace=False, _trace_kwargs=None):
    nc = _get_built()
    in_maps = _host_prep(states, log_tau)
    res = bass_utils.run_bass_kernel_spmd(
        nc, in_maps, core_ids=list(range(NCORES)),
        trace=_trace, **(_trace_kwargs or {}),
    )
    out = np.concatenate([res.results[c]["out"] for c in range(NCORES)], axis=0)
    if _trace:
        kernel._last_results = res
    return out.astype(np.float32)
